# revision 1
# baseline (speedup 1.0000x reference)
"""Multi-head attention (B=2, S=2048, E=1024, H=16) on 8 Trainium2 NeuronCores.

Sharding: core c handles batch b=c//4 and head group g=c%4 (4 heads each).
hidden_states[b] is replicated to the 4 cores of batch b (pre-transposed and
cast to bf16 on host so the contraction dim E lands on SBUF partitions with
plain contiguous DMAs). Each core computes q/k/v projections for its heads,
transposed-layout attention (scores^T = k q'^T so softmax reduces over the
PSUM partition dim via a ones-matmul), and a partial output projection over
its 256 E-dims. The host sums the 4 partials per batch and adds bo.

Bias handling: softmax over t is invariant to per-query constants, so the
k-bias drops out entirely and the q-bias is folded into q' = q + bq. The
v-bias is a post-softmax additive constant (softmax rows sum to 1), applied
after normalization. bo is added on host.
"""

import sys

if "/opt/trn_rl_repo" not in sys.path:
    sys.path.insert(0, "/opt/trn_rl_repo")

import numpy as np
import ml_dtypes

import concourse.bass as bass
import concourse.tile as tile
from concourse import mybir
from concourse.bass_utils import run_bass_kernel_spmd
from concourse.vector_clock import ScopedClock

B, S, E, H = 2, 2048, 1024, 16
DH = E // H  # 64
N_CORES = 8
HEADS_PER_CORE = 4  # 2 pairs
EL = HEADS_PER_CORE * DH  # 256 local E-dims per core

F32 = mybir.dt.float32
BF16 = mybir.dt.bfloat16
BF16_NP = ml_dtypes.bfloat16

ST = 512  # s_tile width (softmax free dim per psum bank)
N_ST = S // ST  # 4
N_TC = S // 128  # 16 t-chunks
N_EC = E // 128  # 8 e-chunks


def _patch_tail_drain():
    """walrus CoreV3 setupSyncWait allows only 1 sem wait on an SP Drain; Tile's
    kernel-tail drain carries one wait per live processor. Split the waits
    across consecutive drains (mutating via nc.inst_map, whose objects are what
    to_json_bytes serializes)."""
    if getattr(tile.TileContext, "_drain_patched", False):
        return

    def _drain_and_barrier(self, tick_clock, wait_clock):
        nc = self.nc
        drain_inst = nc.sync.drain()
        wait_clock.add_sem_waits(
            drain_inst.ins, ScopedClock({None: tick_clock.global_clock})
        )
        inst = nc.inst_map[drain_inst.ins.name]
        w = list(inst.sync_info.on_wait) if inst.sync_info else []
        if len(w) > 1:
            si = inst.sync_info
            si.on_wait = w[:1]
            inst.sync_info = si
            for i in range(1, len(w)):
                d2 = nc.sync.drain()
                i2 = nc.inst_map[d2.ins.name]
                si2 = i2.sync_info or mybir.SyncInfo(on_wait=[], on_update=[])
                si2.on_wait = [w[i]]
                i2.sync_info = si2
        nc.all_engine_barrier()
        assert self.sems is not None
        popped = nc._tile_sem_poison_stack.pop()
        assert popped is self._sem_poison
        nc.clear_and_free_semaphores(list(self.sems.allocated().values()))
        nc.all_engine_barrier()

    tile.TileContext._drain_and_barrier = _drain_and_barrier
    tile.TileContext._drain_patched = True


def _split_multi_waits(nc):
    """The walrus build in this environment accepts only ONE sem-wait command
    per instruction, but Tile's wait-assignment attaches several. Hoist excess
    waits onto dedicated same-engine no-op carrier instructions inserted
    immediately before the owner (same engine-stream position, identical
    semantics)."""
    f = nc.m.functions[0]
    blocks = list(f.blocks)
    carriers: dict[str, list] = {}
    created = set()
    for blk in blocks:
        for inst in blk.instructions:
            if inst.sync_info and len(inst.sync_info.on_wait) > 1:
                w = list(inst.sync_info.on_wait)
                cs = []
                for wx in w[:-1]:
                    # engine nop() appends to nc.cur_bb; it is re-homed below
                    nop = nc.engines[inst.engine].nop(nofuse=True).ins
                    nop.sync_info = mybir.SyncInfo(on_wait=[wx], on_update=[])
                    cs.append(nop)
                    created.add(nop.name)
                si = inst.sync_info
                si.on_wait = [w[-1]]
                inst.sync_info = si
                carriers[inst.name] = cs
    if not carriers:
        return
    for blk in blocks:
        rebuilt = []
        for i in blk.instructions:
            if i.name in created:
                continue
            rebuilt.extend(carriers.get(i.name, ()))
            rebuilt.append(i)
        blk.instructions = rebuilt


def build_bass():
    """Build the per-core Bass program (identical on all 8 cores)."""
    _patch_tail_drain()
    nc = bass.Bass("TRN2", target_bir_lowering=False, debug=False)

    xt_d = nc.dram_tensor("xt", [E, S], BF16, kind="ExternalInput").ap()
    wq_d = nc.dram_tensor("wq", [E, EL], BF16, kind="ExternalInput").ap()
    wk_d = nc.dram_tensor("wk", [E, EL], BF16, kind="ExternalInput").ap()
    wv_d = nc.dram_tensor("wv", [E, EL], BF16, kind="ExternalInput").ap()
    wo_d = nc.dram_tensor("wo", [EL, E], BF16, kind="ExternalInput").ap()
    bq_d = nc.dram_tensor("bq2", [128, 2], F32, kind="ExternalInput").ap()
    bv_d = nc.dram_tensor("bv2", [128, 2], F32, kind="ExternalInput").ap()
    out_d = nc.dram_tensor("out", [S, E], F32, kind="ExternalOutput").ap()

    EXP = mybir.ActivationFunctionType.Exp
    ADD = mybir.AluOpType.add
    MULT = mybir.AluOpType.mult

    with tile.TileContext(nc) as tc:
        with (
            tc.tile_pool(name="const", bufs=1) as const_pool,
            tc.tile_pool(name="xw", bufs=1) as xw_pool,
            tc.tile_pool(name="qkv", bufs=1) as qkv_pool,
            tc.tile_pool(name="exps", bufs=3) as exp_pool,
            tc.tile_pool(name="ctxn", bufs=4) as ctxn_pool,
            tc.tile_pool(name="small", bufs=4) as small_pool,
            tc.tile_pool(name="rb", bufs=4) as rb_pool,
            tc.tile_pool(name="outs", bufs=3) as out_pool,
            tc.tile_pool(name="pp", bufs=2, space="PSUM") as pp_ps,
            tc.tile_pool(name="sc", bufs=2, space="PSUM") as sc_ps,
            tc.tile_pool(name="ctx", bufs=1, space="PSUM") as ctx_ps_pool,
            tc.tile_pool(name="den", bufs=1, space="PSUM") as den_ps_pool,
        ):
            # ---- constants and weights
            ones_sb = const_pool.tile([128, 1], BF16)
            nc.vector.memset(ones_sb[:], 1.0)
            ones1_sb = const_pool.tile([1, 64], mybir.dt.float16)
            nc.vector.memset(ones1_sb[:], 1.0)
            bq_sb = const_pool.tile([128, 2], F32)
            nc.sync.dma_start(bq_sb[:], bq_d[:])
            bv_sb = const_pool.tile([128, 2], F32)
            nc.sync.dma_start(bv_sb[:], bv_d[:])

            wq_sb = xw_pool.tile([128, N_EC, EL], BF16)
            nc.sync.dma_start(wq_sb[:], wq_d.rearrange("(o p) d -> p o d", p=128))
            wk_sb = xw_pool.tile([128, N_EC, EL], BF16)
            nc.sync.dma_start(wk_sb[:], wk_d.rearrange("(o p) d -> p o d", p=128))
            wv_sb = xw_pool.tile([128, N_EC, EL], BF16)
            nc.sync.dma_start(wv_sb[:], wv_d.rearrange("(o p) d -> p o d", p=128))
            wo_sb = xw_pool.tile([128, 2, E], BF16)
            nc.sync.dma_start(wo_sb[:], wo_d.rearrange("(o p) n -> p o n", p=128))

            xt_sb = xw_pool.tile([128, N_EC, S], BF16)
            for ec in range(N_EC):
                nc.sync.dma_start(xt_sb[:, ec, :], xt_d[128 * ec : 128 * (ec + 1), :])

            # ---- projections: q'^T (with bias), k^T, v (natural layout)
            qT = [qkv_pool.tile([128, S], BF16, name=f"qT{p}") for p in range(2)]
            kT = [qkv_pool.tile([128, S], BF16, name=f"kT{p}") for p in range(2)]
            v_sb = qkv_pool.tile([128, N_TC, EL], BF16)

            for p in range(2):
                dlo, dhi = 128 * p, 128 * (p + 1)
                for st in range(N_ST):
                    slo, shi = ST * st, ST * (st + 1)
                    ps_q = pp_ps.tile([128, ST], F32, tag="pp")
                    for ec in range(N_EC):
                        nc.tensor.matmul(
                            ps_q[:],
                            wq_sb[:, ec, dlo:dhi],
                            xt_sb[:, ec, slo:shi],
                            start=(ec == 0),
                            stop=(ec == N_EC - 1),
                        )
                    nc.vector.tensor_scalar(
                        qT[p][:, slo:shi], ps_q[:], bq_sb[:, p : p + 1], None, ADD
                    )
                    ps_k = pp_ps.tile([128, ST], F32, tag="pp")
                    for ec in range(N_EC):
                        nc.tensor.matmul(
                            ps_k[:],
                            wk_sb[:, ec, dlo:dhi],
                            xt_sb[:, ec, slo:shi],
                            start=(ec == 0),
                            stop=(ec == N_EC - 1),
                        )
                    nc.vector.tensor_copy(kT[p][:, slo:shi], ps_k[:])

            for tt in range(N_TC):
                ps_v = pp_ps.tile([128, ST], F32, tag="pp")
                for ec in range(N_EC):
                    nc.tensor.matmul(
                        ps_v[:, :EL],
                        xt_sb[:, ec, 128 * tt : 128 * (tt + 1)],
                        wv_sb[:, ec, :],
                        start=(ec == 0),
                        stop=(ec == N_EC - 1),
                    )
                nc.vector.tensor_copy(v_sb[:, tt, :], ps_v[:, :EL])

            # ---- attention + output projection, per s_tile
            for st in range(N_ST):
                slo, shi = ST * st, ST * (st + 1)
                cns = []
                for p in range(2):
                    ctx_ps = ctx_ps_pool.tile([128, ST], F32)
                    den_ps = den_ps_pool.tile([128, ST], F32)
                    for tc in range(N_TC):
                        tlo, thi = 128 * tc, 128 * (tc + 1)
                        sc = sc_ps.tile([128, 2 * ST], F32)
                        nc.tensor.matmul(
                            sc[:, :ST],
                            kT[p][0:64, tlo:thi],
                            qT[p][0:64, slo:shi],
                            start=True,
                            stop=True,
                        )
                        nc.tensor.matmul(
                            sc[:, ST:],
                            kT[p][64:128, tlo:thi],
                            qT[p][64:128, slo:shi],
                            start=True,
                            stop=True,
                        )
                        ex = exp_pool.tile([128, 2 * ST], BF16)
                        nc.scalar.activation(ex[:], sc[:], EXP, scale=0.125)
                        first, last = tc == 0, tc == N_TC - 1
                        nc.tensor.matmul(
                            ctx_ps[0:64, :],
                            v_sb[:, tc, 128 * p : 128 * p + 64],
                            ex[:, :ST],
                            start=first,
                            stop=last,
                        )
                        nc.tensor.matmul(
                            ctx_ps[64:128, :],
                            v_sb[:, tc, 128 * p + 64 : 128 * (p + 1)],
                            ex[:, ST:],
                            start=first,
                            stop=last,
                        )
                        nc.tensor.matmul(
                            den_ps[0:1, :], ones_sb[:], ex[:, :ST],
                            start=first, stop=last,
                        )
                        nc.tensor.matmul(
                            den_ps[64:65, :], ones_sb[:], ex[:, ST:],
                            start=first, stop=last,
                        )
                    # normalize: ctx / denom + bv  (denom recip rows broadcast
                    # across the 64 head partitions via a stride-0 DMA read)
                    r0 = small_pool.tile([1, ST], mybir.dt.float16, tag="r0")
                    r1 = small_pool.tile([1, ST], mybir.dt.float16, tag="r1")
                    with nc.allow_low_precision(
                        reason="fp16 reciprocal rows: 5e-4 rel err, well under bf16 ctx"
                    ):
                        nc.vector.reciprocal(r0[:], den_ps[0:1, :])
                        nc.vector.reciprocal(r1[:], den_ps[64:65, :])
                    rbp = pp_ps.tile([128, ST], F32, tag="pp")
                    nc.tensor.matmul(
                        rbp[0:64, :],
                        ones1_sb[:],
                        r0[:],
                        start=True,
                        stop=True,
                    )
                    nc.tensor.matmul(
                        rbp[64:128, :],
                        ones1_sb[:],
                        r1[:],
                        start=True,
                        stop=True,
                    )
                    rb = rb_pool.tile([128, ST], F32, tag="rb")
                    nc.vector.tensor_copy(rb[:], rbp[:])
                    cn = ctxn_pool.tile([128, ST], BF16)
                    nc.vector.tensor_tensor(
                        cn[0:64, :], ctx_ps[0:64, :], rb[0:64, :], MULT
                    )
                    nc.vector.tensor_tensor(
                        cn[64:128, :], ctx_ps[64:128, :], rb[64:128, :], MULT
                    )
                    nc.vector.tensor_scalar(
                        cn[:], cn[:], bv_sb[:, p : p + 1], None, ADD
                    )
                    cns.append(cn)
                # output projection for this s_tile
                for ss in range(ST // 128):
                    srow = slo + 128 * ss
                    for nt in range(E // ST):
                        ps_o = pp_ps.tile([128, ST], F32, tag="pp")
                        nc.tensor.matmul(
                            ps_o[:],
                            cns[0][:, 128 * ss : 128 * (ss + 1)],
                            wo_sb[:, 0, ST * nt : ST * (nt + 1)],
                            start=True,
                            stop=False,
                        )
                        nc.tensor.matmul(
                            ps_o[:],
                            cns[1][:, 128 * ss : 128 * (ss + 1)],
                            wo_sb[:, 1, ST * nt : ST * (nt + 1)],
                            start=False,
                            stop=True,
                        )
                        ob = out_pool.tile([128, ST], F32)
                        nc.vector.tensor_copy(ob[:], ps_o[:])
                        nc.sync.dma_start(
                            out_d[srow : srow + 128, ST * nt : ST * (nt + 1)], ob[:]
                        )
    _split_multi_waits(nc)
    return nc


_NC = None


def _get_nc():
    global _NC
    if _NC is None:
        _NC = build_bass()
    return _NC


def make_in_maps(hidden_states, Wq, bq, Wk, bk, Wv, bv, Wo):
    """Host-side sharding/layout prep. Returns list of 8 per-core input dicts."""
    hs = np.asarray(hidden_states, dtype=np.float32)
    Wq = np.asarray(Wq, dtype=np.float32)
    Wk = np.asarray(Wk, dtype=np.float32)
    Wv = np.asarray(Wv, dtype=np.float32)
    Wo = np.asarray(Wo, dtype=np.float32)
    bq = np.asarray(bq, dtype=np.float32)
    bv = np.asarray(bv, dtype=np.float32)

    xt = [
        np.ascontiguousarray(hs[b].T).astype(BF16_NP) for b in range(B)
    ]  # [E, S] bf16
    in_maps = []
    for c in range(N_CORES):
        b, g = divmod(c, N_CORES // B)
        h0 = HEADS_PER_CORE * g
        hsl = slice(h0, h0 + HEADS_PER_CORE)
        # [H_loc, E, DH] -> [E, H_loc*DH] head-major columns
        wq_c = np.ascontiguousarray(
            Wq[hsl].transpose(1, 0, 2).reshape(E, EL)
        ).astype(BF16_NP)
        wk_c = np.ascontiguousarray(
            Wk[hsl].transpose(1, 0, 2).reshape(E, EL)
        ).astype(BF16_NP)
        wv_c = np.ascontiguousarray(
            Wv[hsl].transpose(1, 0, 2).reshape(E, EL)
        ).astype(BF16_NP)
        wo_c = np.ascontiguousarray(Wo[EL * g : EL * (g + 1), :]).astype(BF16_NP)
        bq_c = np.ascontiguousarray(bq[hsl].reshape(EL).reshape(2, 128).T)
        bv_c = np.ascontiguousarray(bv[hsl].reshape(EL).reshape(2, 128).T)
        in_maps.append(
            {
                "xt": xt[b],
                "wq": wq_c,
                "wk": wk_c,
                "wv": wv_c,
                "wo": wo_c,
                "bq2": bq_c,
                "bv2": bv_c,
            }
        )
    return in_maps


def kernel(hidden_states, mask, Wq, bq, Wk, bk, Wv, bv, Wo, bo, **run_kwargs):
    """Full-input entry point. mask is all-ones per the problem spec (ignored)."""
    nc = _get_nc()
    in_maps = make_in_maps(hidden_states, Wq, bq, Wk, bk, Wv, bv, Wo)
    res = run_bass_kernel_spmd(nc, in_maps, core_ids=list(range(N_CORES)), **run_kwargs)
    bo = np.asarray(bo, dtype=np.float32)
    out = np.zeros((B, S, E), dtype=np.float32)
    for c in range(N_CORES):
        out[c // (N_CORES // B)] += res.results[c]["out"]
    out += bo
    kernel.last_results = res
    return out



# revision 2
# speedup vs baseline: 1.5238x; 1.5238x over previous
"""Multi-head attention (B=2, S=2048, E=1024, H=16) on 8 Trainium2 NeuronCores.

Sharding: core c handles batch b=c//4 and head group g=c%4 (4 heads each).
hidden_states[b] is replicated to the 4 cores of batch b (pre-transposed and
cast to bf16 on host so the contraction dim E lands on SBUF partitions with
plain contiguous DMAs). Each core computes q/k/v projections for its heads,
transposed-layout attention, and a partial output projection over its 256
E-dims. The host sums the 4 partials per batch and adds bo.

Schedule: a single software-pipelined stream. Projections (k/q/v chains) are
injected into the first s-tile's attention iterations so the exp stream on the
Scalar engine starts ~8us in and the PE never idles long enough for the HAM
clock gate to re-throttle. The softmax denominator is fused into the ctx
matmul: head A's stationary is [vA(64) | ones | 0*63] (ctx at psum partitions
0:64, denom at 64), head B's is [ones | 0*63 | vB(64)] (denom at partition 0,
ctx at 64:128) so all downstream DVE ops are partition-aligned and the output
projection keeps C=128 contraction. Normalization: denominators are copied to
fp16, broadcast over 64 partitions with a tiny ones-matmul, reciprocal'd once
per [128,512] tile, multiplied into cn, bias-added.

Bias handling: softmax over t is invariant to per-query constants, so the
k-bias drops out and the q-bias folds into q. The v-bias is a post-softmax
additive constant (softmax rows sum to 1). bo is added on host.
"""

import sys

if "/opt/trn_rl_repo" not in sys.path:
    sys.path.insert(0, "/opt/trn_rl_repo")

import numpy as np
import ml_dtypes

import concourse.bass as bass
import concourse.tile as tile
from concourse import mybir
from concourse.bass_utils import run_bass_kernel_spmd
from concourse.vector_clock import ScopedClock

B, S, E, H = 2, 2048, 1024, 16
DH = E // H  # 64
N_CORES = 8
HEADS_PER_CORE = 4  # 2 pairs
EL = HEADS_PER_CORE * DH  # 256 local E-dims per core

F32 = mybir.dt.float32
BF16 = mybir.dt.bfloat16
FP16 = mybir.dt.float16
BF16_NP = ml_dtypes.bfloat16

ST = 512  # s_tile width
N_ST = S // ST  # 4
N_TC = S // 128  # 16 t-chunks
N_EC = E // 128  # 8 e-chunks
LAG = 4  # ctx matmuls trail scores/exp by this many iterations


def _patch_tail_drain():
    """walrus CoreV3 setupSyncWait allows only 1 sem wait on an SP Drain; Tile's
    kernel-tail drain carries one wait per live processor. Split the waits
    across consecutive drains (mutating via nc.inst_map, whose objects are what
    to_json_bytes serializes)."""
    if getattr(tile.TileContext, "_drain_patched", False):
        return

    def _drain_and_barrier(self, tick_clock, wait_clock):
        nc = self.nc
        drain_inst = nc.sync.drain()
        wait_clock.add_sem_waits(
            drain_inst.ins, ScopedClock({None: tick_clock.global_clock})
        )
        inst = nc.inst_map[drain_inst.ins.name]
        w = list(inst.sync_info.on_wait) if inst.sync_info else []
        if len(w) > 1:
            si = inst.sync_info
            si.on_wait = w[:1]
            inst.sync_info = si
            for i in range(1, len(w)):
                d2 = nc.sync.drain()
                i2 = nc.inst_map[d2.ins.name]
                si2 = i2.sync_info or mybir.SyncInfo(on_wait=[], on_update=[])
                si2.on_wait = [w[i]]
                i2.sync_info = si2
        nc.all_engine_barrier()
        assert self.sems is not None
        popped = nc._tile_sem_poison_stack.pop()
        assert popped is self._sem_poison
        nc.clear_and_free_semaphores(list(self.sems.allocated().values()))
        nc.all_engine_barrier()

    tile.TileContext._drain_and_barrier = _drain_and_barrier
    tile.TileContext._drain_patched = True


def _split_multi_waits(nc):
    """The walrus build in this environment accepts only ONE sem-wait command
    per instruction, but Tile's wait-assignment attaches several. Hoist excess
    waits onto dedicated same-engine no-op carrier instructions inserted
    immediately before the owner (same engine-stream position, identical
    semantics)."""
    f = nc.m.functions[0]
    blocks = list(f.blocks)
    carriers: dict[str, list] = {}
    created = set()
    for blk in blocks:
        for inst in blk.instructions:
            if inst.sync_info and len(inst.sync_info.on_wait) > 1:
                w = list(inst.sync_info.on_wait)
                cs = []
                for wx in w[:-1]:
                    # engine nop() appends to nc.cur_bb; it is re-homed below
                    nop = nc.engines[inst.engine].nop(nofuse=True).ins
                    nop.sync_info = mybir.SyncInfo(on_wait=[wx], on_update=[])
                    cs.append(nop)
                    created.add(nop.name)
                si = inst.sync_info
                si.on_wait = [w[-1]]
                inst.sync_info = si
                carriers[inst.name] = cs
    if not carriers:
        return
    for blk in blocks:
        rebuilt = []
        for i in blk.instructions:
            if i.name in created:
                continue
            rebuilt.extend(carriers.get(i.name, ()))
            rebuilt.append(i)
        blk.instructions = rebuilt


def build_bass():
    """Build the per-core Bass program (identical on all 8 cores)."""
    _patch_tail_drain()
    nc = bass.Bass("TRN2", target_bir_lowering=False, debug=False)

    xt_d = nc.dram_tensor("xt", [E, S], BF16, kind="ExternalInput").ap()
    wq_d = nc.dram_tensor("wq", [E, EL], BF16, kind="ExternalInput").ap()
    wk_d = nc.dram_tensor("wk", [E, EL], BF16, kind="ExternalInput").ap()
    wv_d = nc.dram_tensor("wv", [E, EL], BF16, kind="ExternalInput").ap()
    wo_d = nc.dram_tensor("wo", [EL, E], BF16, kind="ExternalInput").ap()
    bq_d = nc.dram_tensor("bq2", [128, 2], F32, kind="ExternalInput").ap()
    bv_d = nc.dram_tensor("bv2", [128, 2], F32, kind="ExternalInput").ap()
    out_d = nc.dram_tensor("out", [S, E], F32, kind="ExternalOutput").ap()

    EXP = mybir.ActivationFunctionType.Exp
    ADD = mybir.AluOpType.add
    MULT = mybir.AluOpType.mult

    with tile.TileContext(nc) as tc:
        with (
            tc.tile_pool(name="const", bufs=1) as const_pool,
            tc.tile_pool(name="xw", bufs=1) as xw_pool,
            tc.tile_pool(name="qkv", bufs=1) as qkv_pool,
            tc.tile_pool(name="exs", bufs=8) as ex_pool,
            tc.tile_pool(name="cns", bufs=3) as cn_pool,
            tc.tile_pool(name="rb32", bufs=2) as rb32_pool,
            tc.tile_pool(name="rbh", bufs=2) as rbh_pool,
            tc.tile_pool(name="dh", bufs=4) as dh_pool,
            tc.tile_pool(name="outs", bufs=3) as out_pool,
            tc.tile_pool(name="psa", bufs=2, space="PSUM") as psa,
            tc.tile_pool(name="psc", bufs=4, space="PSUM") as psc,
        ):
            # ---- constants and weights
            ones1 = const_pool.tile([1, 64], FP16)
            nc.vector.memset(ones1[:], 1.0)
            bq_sb = const_pool.tile([128, 2], F32)
            nc.sync.dma_start(bq_sb[:], bq_d[:])
            bv_sb = const_pool.tile([128, 2], F32)
            nc.sync.dma_start(bv_sb[:], bv_d[:])

            wq_sb = xw_pool.tile([128, N_EC, EL], BF16)
            nc.sync.dma_start(wq_sb[:], wq_d.rearrange("(o p) d -> p o d", p=128))
            wk_sb = xw_pool.tile([128, N_EC, EL], BF16)
            nc.sync.dma_start(wk_sb[:], wk_d.rearrange("(o p) d -> p o d", p=128))
            wv_sb = xw_pool.tile([128, N_EC, EL], BF16)
            nc.sync.dma_start(wv_sb[:], wv_d.rearrange("(o p) d -> p o d", p=128))
            wo_sb = xw_pool.tile([128, 2, E], BF16)
            nc.sync.dma_start(wo_sb[:], wo_d.rearrange("(o p) n -> p o n", p=128))

            # xt by s-quarters so the first k/q chains start early
            xt_sb = xw_pool.tile([128, N_EC, S], BF16)
            for q4 in range(4):
                for ec in range(N_EC):
                    nc.sync.dma_start(
                        xt_sb[:, ec, 512 * q4 : 512 * (q4 + 1)],
                        xt_d[128 * ec : 128 * (ec + 1), 512 * q4 : 512 * (q4 + 1)],
                    )

            # ---- persistent SBUF tensors
            qT = [qkv_pool.tile([128, S], BF16, name=f"qT{p}") for p in range(2)]
            kT = [qkv_pool.tile([128, S], BF16, name=f"kT{p}") for p in range(2)]
            # v65: per (t-chunk, head) a 128-col stationary.
            #   even head (A): [v(64) | ones | 0*63]  -> ctx @ psum 0:64, den @ 64
            #   odd head (B):  [ones | 0*63 | v(64)]  -> den @ psum 0, ctx @ 64:128
            v65 = qkv_pool.tile([128, N_TC, 4, 128], BF16)
            nc.vector.memset(v65[:], 0.0)
            nc.vector.memset(v65[:, :, 0::2, 64:65], 1.0)
            nc.vector.memset(v65[:, :, 1::2, 0:1], 1.0)

            cn = {}  # (st, p) -> cn tile

            # ---------------- emission closures ----------------
            def k_chain(p, kt):
                ps = psa.tile([128, 2 * ST], F32, tag="a", name="ps_k")
                for ec in range(N_EC):
                    nc.tensor.matmul(
                        ps[:, :ST],
                        wk_sb[:, ec, 128 * p : 128 * (p + 1)],
                        xt_sb[:, ec, ST * kt : ST * (kt + 1)],
                        start=(ec == 0),
                        stop=(ec == N_EC - 1),
                    )
                nc.vector.tensor_copy(kT[p][:, ST * kt : ST * (kt + 1)], ps[:, :ST])

            def q_chain(p, st):
                ps = psa.tile([128, 2 * ST], F32, tag="a", name="ps_q")
                for ec in range(N_EC):
                    nc.tensor.matmul(
                        ps[:, :ST],
                        wq_sb[:, ec, 128 * p : 128 * (p + 1)],
                        xt_sb[:, ec, ST * st : ST * (st + 1)],
                        start=(ec == 0),
                        stop=(ec == N_EC - 1),
                    )
                nc.vector.tensor_scalar(
                    qT[p][:, ST * st : ST * (st + 1)],
                    ps[:, :ST],
                    bq_sb[:, p : p + 1],
                    None,
                    ADD,
                )

            def v_chain(tt):
                ps = psa.tile([128, 2 * ST], F32, tag="a", name="ps_v")
                for ec in range(N_EC):
                    nc.tensor.matmul(
                        ps[:, :EL],
                        xt_sb[:, ec, 128 * tt : 128 * (tt + 1)],
                        wv_sb[:, ec, :],
                        start=(ec == 0),
                        stop=(ec == N_EC - 1),
                    )
                vsrc = ps[:, :EL].rearrange("p (h d) -> p h d", h=4)
                # even heads -> cols 0:64, odd heads -> cols 64:128
                nc.vector.tensor_copy(v65[:, tt, 0::2, 0:64], vsrc[:, 0::2, :])
                nc.vector.tensor_copy(v65[:, tt, 1::2, 64:128], vsrc[:, 1::2, :])

            ctx_ps = {}  # (st, p) -> (ctxA tile, ctxB tile)
            ex_tiles = {}  # (st, p, tc) -> ex tile (deleted after use)

            def scores(st, p, tcc):
                sc = psa.tile([128, 2 * ST], F32, tag="a", name="sc")
                nc.tensor.matmul(
                    sc[:, :ST],
                    kT[p][0:64, 128 * tcc : 128 * (tcc + 1)],
                    qT[p][0:64, ST * st : ST * (st + 1)],
                    start=True,
                    stop=True,
                )
                nc.tensor.matmul(
                    sc[:, ST:],
                    kT[p][64:128, 128 * tcc : 128 * (tcc + 1)],
                    qT[p][64:128, ST * st : ST * (st + 1)],
                    start=True,
                    stop=True,
                )
                return sc

            def exp_emit(st, p, tcc, sc):
                ex = ex_pool.tile([128, 2 * ST], BF16, name="ex")
                nc.scalar.activation(ex[:], sc[:], EXP, scale=0.125)
                ex_tiles[(st, p, tcc)] = ex

            def ctx_pair(st, p, tcc):
                if (st, p) not in ctx_ps:
                    a = psc.tile([128, ST], F32, tag="c", name="ctxA")
                    b = psc.tile([128, ST], F32, tag="c", name="ctxB")
                    ctx_ps[(st, p)] = (a, b)
                a, b = ctx_ps[(st, p)]
                ex = ex_tiles.pop((st, p, tcc))
                first, last = tcc == 0, tcc == N_TC - 1
                nc.tensor.matmul(
                    a[:], v65[:, tcc, 2 * p, :], ex[:, :ST], start=first, stop=last
                )
                nc.tensor.matmul(
                    b[:], v65[:, tcc, 2 * p + 1, :], ex[:, ST:], start=first, stop=last
                )

            def den_copies(st, p):
                a, b = ctx_ps[(st, p)]
                dAh = dh_pool.tile([1, ST], FP16, tag="dh", name="dAh")
                nc.vector.tensor_copy(dAh[:], a[64:65, :])
                dBh = dh_pool.tile([1, ST], FP16, tag="dh", name="dBh")
                nc.vector.tensor_copy(dBh[:], b[0:1, :])
                ctx_ps[(st, p)] = (a, b, dAh, dBh)

            def norm(st, p):
                a, b, dAh, dBh = ctx_ps.pop((st, p))
                rbp = psa.tile([128, 2 * ST], F32, tag="a", name="rbp")
                nc.tensor.matmul(
                    rbp[0:64, :ST], ones1[:], dAh[:], start=True, stop=True
                )
                nc.tensor.matmul(
                    rbp[64:128, :ST], ones1[:], dBh[:], start=True, stop=True
                )
                rb32 = rb32_pool.tile([128, ST], F32, name="rb32")
                nc.vector.tensor_copy(rb32[:], rbp[:, :ST])
                rbh = rbh_pool.tile([128, ST], FP16, name="rbh")
                with nc.allow_low_precision(reason="fp16 recip of softmax denom"):
                    nc.vector.reciprocal(rbh[:], rb32[:])
                c = cn_pool.tile([128, ST], BF16, name="cn")
                nc.vector.tensor_tensor(c[0:64, :], a[0:64, :], rbh[0:64, :], MULT)
                nc.vector.tensor_tensor(
                    c[64:128, :], b[64:128, :], rbh[64:128, :], MULT
                )
                nc.vector.tensor_scalar(c[:], c[:], bv_sb[:, p : p + 1], None, ADD)
                cn[(st, p)] = c

            def outproj(st, j):
                ss, nt = j // 2, j % 2
                ps = psa.tile([128, 2 * ST], F32, tag="a", name="ps_o")
                nc.tensor.matmul(
                    ps[:, :ST],
                    cn[(st, 0)][:, 128 * ss : 128 * (ss + 1)],
                    wo_sb[:, 0, ST * nt : ST * (nt + 1)],
                    start=True,
                    stop=False,
                )
                nc.tensor.matmul(
                    ps[:, :ST],
                    cn[(st, 1)][:, 128 * ss : 128 * (ss + 1)],
                    wo_sb[:, 1, ST * nt : ST * (nt + 1)],
                    start=False,
                    stop=True,
                )
                ob = out_pool.tile([128, ST], F32, name="ob")
                nc.vector.tensor_copy(ob[:], ps[:, :ST])
                srow = ST * st + 128 * ss
                nc.sync.dma_start(
                    out_d[srow : srow + 128, ST * nt : ST * (nt + 1)], ob[:]
                )

            # ---------------- schedule ----------------
            # Injections per loop (st, p), keyed by iteration index.
            def make_fillers():
                F = {(st, p): {i: [] for i in range(N_TC)} for st in range(N_ST)
                     for p in range(2)}
                # kv/q production spread over st0
                F[(0, 0)][0] += [lambda: k_chain(0, 1), lambda: v_chain(1)]
                for i, tt in [(1, 2), (2, 3), (3, 4)]:
                    F[(0, 0)][i] += [lambda t=tt: v_chain(t)]
                F[(0, 0)][4] += [lambda: k_chain(0, 2), lambda: v_chain(5)]
                for i, tt in [(5, 6), (6, 7), (7, 8)]:
                    F[(0, 0)][i] += [lambda t=tt: v_chain(t)]
                F[(0, 0)][8] += [lambda: k_chain(0, 3), lambda: v_chain(9)]
                for i, tt in [(9, 10), (10, 11)]:
                    F[(0, 0)][i] += [lambda t=tt: v_chain(t)]
                F[(0, 0)][11] += [lambda: k_chain(1, 0), lambda: v_chain(12)]
                F[(0, 0)][12] += [lambda: q_chain(1, 0), lambda: v_chain(13)]
                F[(0, 0)][13] += [lambda: k_chain(1, 1), lambda: v_chain(14)]
                F[(0, 0)][14] += [lambda: v_chain(15)]
                F[(0, 1)][0] += [lambda: k_chain(1, 2)]
                F[(0, 1)][2] += [lambda: k_chain(1, 3)]
                # q for next s-tile, emitted in each p1 loop
                for st in range(N_ST - 1):
                    F[(st, 1)][10] += [lambda s=st: q_chain(0, s + 1)]
                    F[(st, 1)][11] += [lambda s=st: q_chain(1, s + 1)]
                # output projection of s-tile st-1, spread over the (st, 0) loop
                for st in range(1, N_ST):
                    for j in range(8):
                        F[(st, 0)][8 + j] += [lambda s=st - 1, jj=j: outproj(s, jj)]
                return F

            fillers = make_fillers()
            loops = [(st, p) for st in range(N_ST) for p in range(2)]

            # preamble
            k_chain(0, 0)
            q_chain(0, 0)
            v_chain(0)

            carry = []  # closures to inject at the start of the next loop
            for li, (st, p) in enumerate(loops):
                my_fill = fillers[(st, p)]
                for i in range(N_TC):
                    sc = scores(st, p, i)
                    # carried work from the previous loop: ctx tail + den + norm
                    if i < len(carry):
                        carry[i]()
                    for f in my_fill[i]:
                        f()
                    exp_emit(st, p, i, sc)
                    if i >= LAG:
                        ctx_pair(st, p, i - LAG)
                # build next carry: finish this loop's ctx, den, then norm;
                # plus (in p0 loops) the output projection of s-tile st-1
                nxt = []
                for tcc in range(N_TC - LAG, N_TC):
                    nxt.append(lambda t=tcc, s=st, q=p: ctx_pair(s, q, t))
                nxt.append(lambda s=st, q=p: den_copies(s, q))
                nxt.append(lambda s=st, q=p: norm(s, q))
                carry = nxt

            # tail: flush the last carry (ctx tail, den, norm of (3,1)),
            # then the final output projection
            for f in carry:
                f()
            for j in range(8):
                outproj(N_ST - 1, j)
    _split_multi_waits(nc)
    return nc


_NC = None


def _get_nc():
    global _NC
    if _NC is None:
        _NC = build_bass()
    return _NC


def make_in_maps(hidden_states, Wq, bq, Wk, bk, Wv, bv, Wo):
    """Host-side sharding/layout prep. Returns list of 8 per-core input dicts."""
    hs = np.asarray(hidden_states, dtype=np.float32)
    Wq = np.asarray(Wq, dtype=np.float32)
    Wk = np.asarray(Wk, dtype=np.float32)
    Wv = np.asarray(Wv, dtype=np.float32)
    Wo = np.asarray(Wo, dtype=np.float32)
    bq = np.asarray(bq, dtype=np.float32)
    bv = np.asarray(bv, dtype=np.float32)

    xt = [
        np.ascontiguousarray(hs[b].T).astype(BF16_NP) for b in range(B)
    ]  # [E, S] bf16
    in_maps = []
    for c in range(N_CORES):
        b, g = divmod(c, N_CORES // B)
        h0 = HEADS_PER_CORE * g
        hsl = slice(h0, h0 + HEADS_PER_CORE)
        # [H_loc, E, DH] -> [E, H_loc*DH] head-major columns
        wq_c = np.ascontiguousarray(
            Wq[hsl].transpose(1, 0, 2).reshape(E, EL)
        ).astype(BF16_NP)
        wk_c = np.ascontiguousarray(
            Wk[hsl].transpose(1, 0, 2).reshape(E, EL)
        ).astype(BF16_NP)
        wv_c = np.ascontiguousarray(
            Wv[hsl].transpose(1, 0, 2).reshape(E, EL)
        ).astype(BF16_NP)
        wo_c = np.ascontiguousarray(Wo[EL * g : EL * (g + 1), :]).astype(BF16_NP)
        bq_c = np.ascontiguousarray(bq[hsl].reshape(EL).reshape(2, 128).T)
        bv_c = np.ascontiguousarray(bv[hsl].reshape(EL).reshape(2, 128).T)
        in_maps.append(
            {
                "xt": xt[b],
                "wq": wq_c,
                "wk": wk_c,
                "wv": wv_c,
                "wo": wo_c,
                "bq2": bq_c,
                "bv2": bv_c,
            }
        )
    return in_maps


def kernel(hidden_states, mask, Wq, bq, Wk, bk, Wv, bv, Wo, bo, **run_kwargs):
    """Full-input entry point. mask is all-ones per the problem spec (ignored)."""
    nc = _get_nc()
    in_maps = make_in_maps(hidden_states, Wq, bq, Wk, bk, Wv, bv, Wo)
    res = run_bass_kernel_spmd(nc, in_maps, core_ids=list(range(N_CORES)), **run_kwargs)
    bo = np.asarray(bo, dtype=np.float32)
    out = np.zeros((B, S, E), dtype=np.float32)
    for c in range(N_CORES):
        out[c // (N_CORES // B)] += res.results[c]["out"]
    out += bo
    kernel.last_results = res
    return out


# revision 3
# speedup vs baseline: 1.6122x; 1.0581x over previous
"""Multi-head attention (B=2, S=2048, E=1024, H=16) on 8 Trainium2 NeuronCores.

Sharding: core c handles batch b=c//4 and head group g=c%4 (4 heads each).
hidden_states[b] is replicated to the 4 cores of batch b (pre-transposed and
cast to bf16 on host so the contraction dim E lands on SBUF partitions with
plain contiguous DMAs). Each core computes q/k/v projections for its heads,
transposed-layout attention, and a partial output projection over its 256
E-dims. The host sums the 4 partials per batch and adds bo.

Schedule: a single software-pipelined stream. Projections (k/q/v chains) are
injected into the first s-tile's attention iterations so the exp stream on the
Scalar engine starts ~8us in and the PE never idles long enough for the HAM
clock gate to re-throttle. The softmax denominator is fused into the ctx
matmul: head A's stationary is [vA(64) | ones | 0*63] (ctx at psum partitions
0:64, denom at 64), head B's is [ones | 0*63 | vB(64)] (denom at partition 0,
ctx at 64:128) so all downstream DVE ops are partition-aligned and the output
projection keeps C=128 contraction. Normalization: denominators are copied to
fp16, broadcast over 64 partitions with a tiny ones-matmul, reciprocal'd once
per [128,512] tile, multiplied into cn, bias-added.

Bias handling: softmax over t is invariant to per-query constants, so the
k-bias drops out and the q-bias folds into q. The v-bias is a post-softmax
additive constant (softmax rows sum to 1). bo is added on host.
"""

import sys

if "/opt/trn_rl_repo" not in sys.path:
    sys.path.insert(0, "/opt/trn_rl_repo")

import numpy as np
import ml_dtypes

import concourse.bass as bass
import concourse.tile as tile
from concourse import mybir
from concourse.bass_utils import run_bass_kernel_spmd
from concourse.vector_clock import ScopedClock

B, S, E, H = 2, 2048, 1024, 16
DH = E // H  # 64
N_CORES = 8
HEADS_PER_CORE = 4  # 2 pairs
EL = HEADS_PER_CORE * DH  # 256 local E-dims per core

F32 = mybir.dt.float32
BF16 = mybir.dt.bfloat16
FP16 = mybir.dt.float16
BF16_NP = ml_dtypes.bfloat16

ST = 512  # s_tile width
N_ST = S // ST  # 4
N_TC = S // 128  # 16 t-chunks
N_EC = E // 128  # 8 e-chunks
LAG = 4  # ctx matmuls trail scores/exp by this many iterations


def _patch_tail_drain():
    """walrus CoreV3 setupSyncWait allows only 1 sem wait on an SP Drain; Tile's
    kernel-tail drain carries one wait per live processor. Split the waits
    across consecutive drains (mutating via nc.inst_map, whose objects are what
    to_json_bytes serializes)."""
    if getattr(tile.TileContext, "_drain_patched", False):
        return

    def _drain_and_barrier(self, tick_clock, wait_clock):
        nc = self.nc
        drain_inst = nc.sync.drain()
        wait_clock.add_sem_waits(
            drain_inst.ins, ScopedClock({None: tick_clock.global_clock})
        )
        inst = nc.inst_map[drain_inst.ins.name]
        w = list(inst.sync_info.on_wait) if inst.sync_info else []
        if len(w) > 1:
            si = inst.sync_info
            si.on_wait = w[:1]
            inst.sync_info = si
            for i in range(1, len(w)):
                d2 = nc.sync.drain()
                i2 = nc.inst_map[d2.ins.name]
                si2 = i2.sync_info or mybir.SyncInfo(on_wait=[], on_update=[])
                si2.on_wait = [w[i]]
                i2.sync_info = si2
        nc.all_engine_barrier()
        assert self.sems is not None
        popped = nc._tile_sem_poison_stack.pop()
        assert popped is self._sem_poison
        nc.clear_and_free_semaphores(list(self.sems.allocated().values()))
        nc.all_engine_barrier()

    tile.TileContext._drain_and_barrier = _drain_and_barrier
    tile.TileContext._drain_patched = True


def _split_multi_waits(nc):
    """The walrus build in this environment accepts only ONE sem-wait command
    per instruction, but Tile's wait-assignment attaches several. Hoist excess
    waits onto dedicated same-engine no-op carrier instructions inserted
    immediately before the owner (same engine-stream position, identical
    semantics)."""
    f = nc.m.functions[0]
    blocks = list(f.blocks)
    carriers: dict[str, list] = {}
    created = set()
    for blk in blocks:
        for inst in blk.instructions:
            if inst.sync_info and len(inst.sync_info.on_wait) > 1:
                w = list(inst.sync_info.on_wait)
                cs = []
                for wx in w[:-1]:
                    # engine nop() appends to nc.cur_bb; it is re-homed below
                    nop = nc.engines[inst.engine].nop(nofuse=True).ins
                    nop.sync_info = mybir.SyncInfo(on_wait=[wx], on_update=[])
                    cs.append(nop)
                    created.add(nop.name)
                si = inst.sync_info
                si.on_wait = [w[-1]]
                inst.sync_info = si
                carriers[inst.name] = cs
    if not carriers:
        return
    for blk in blocks:
        rebuilt = []
        for i in blk.instructions:
            if i.name in created:
                continue
            rebuilt.extend(carriers.get(i.name, ()))
            rebuilt.append(i)
        blk.instructions = rebuilt


def build_bass():
    """Build the per-core Bass program (identical on all 8 cores)."""
    _patch_tail_drain()
    nc = bass.Bass("TRN2", target_bir_lowering=False, debug=False)

    xt_d = nc.dram_tensor("xt", [E * S], BF16, kind="ExternalInput").ap()
    wq_d = nc.dram_tensor("wq", [E * EL], BF16, kind="ExternalInput").ap()
    wk_d = nc.dram_tensor("wk", [E * EL], BF16, kind="ExternalInput").ap()
    wv_d = nc.dram_tensor("wv", [E * EL], BF16, kind="ExternalInput").ap()
    wo_d = nc.dram_tensor("wo", [EL * E], BF16, kind="ExternalInput").ap()
    bq_d = nc.dram_tensor("bq2", [128, 2], F32, kind="ExternalInput").ap()
    bv_d = nc.dram_tensor("bv2", [128, 2], F32, kind="ExternalInput").ap()
    out_d = nc.dram_tensor("out", [S, E], BF16, kind="ExternalOutput").ap()

    EXP = mybir.ActivationFunctionType.Exp
    ADD = mybir.AluOpType.add
    MULT = mybir.AluOpType.mult

    with tile.TileContext(nc) as tc:
        with (
            tc.tile_pool(name="const", bufs=1) as const_pool,
            tc.tile_pool(name="xw", bufs=1) as xw_pool,
            tc.tile_pool(name="qkv", bufs=1) as qkv_pool,
            tc.tile_pool(name="exs", bufs=8) as ex_pool,
            tc.tile_pool(name="cns", bufs=3) as cn_pool,
            tc.tile_pool(name="rb32", bufs=2) as rb32_pool,
            tc.tile_pool(name="rbh", bufs=2) as rbh_pool,
            tc.tile_pool(name="dh", bufs=4) as dh_pool,
            tc.tile_pool(name="outs", bufs=3) as out_pool,
            tc.tile_pool(name="psa", bufs=2, space="PSUM") as psa,
            tc.tile_pool(name="psc", bufs=4, space="PSUM") as psc,
        ):
            # ---- constants and weights
            ones1 = const_pool.tile([1, 64], FP16)
            nc.vector.memset(ones1[:], 1.0)
            bq_sb = const_pool.tile([128, 2], F32)
            nc.sync.dma_start(bq_sb[:], bq_d[:])
            bv_sb = const_pool.tile([128, 2], F32)
            nc.sync.dma_start(bv_sb[:], bv_d[:])

            # weights arrive host-pre-transposed to [128, o, d] so every DMA
            # is contiguous 4KB-per-partition; wk/wq + the first xt quarter are
            # emitted first so the k/q chains start within a few us.
            wk_sb = xw_pool.tile([128, N_EC, EL], BF16)
            nc.sync.dma_start(wk_sb[:], wk_d.rearrange("(p o d) -> p o d", p=128, o=N_EC))
            wq_sb = xw_pool.tile([128, N_EC, EL], BF16)
            nc.sync.dma_start(wq_sb[:], wq_d.rearrange("(p o d) -> p o d", p=128, o=N_EC))
            xt_sb = xw_pool.tile([128, N_EC, S], BF16)
            xt_q = xt_d.rearrange("(p q o s) -> p q o s", p=128, q=4, o=N_EC)
            nc.sync.dma_start(xt_sb[:, :, 0:512], xt_q[:, 0])
            wv_sb = xw_pool.tile([128, N_EC, EL], BF16)
            nc.sync.dma_start(wv_sb[:], wv_d.rearrange("(p o d) -> p o d", p=128, o=N_EC))
            for q4 in range(1, 4):
                nc.sync.dma_start(
                    xt_sb[:, :, 512 * q4 : 512 * (q4 + 1)], xt_q[:, q4]
                )
            wo_sb = xw_pool.tile([128, 2, E], BF16)
            nc.sync.dma_start(wo_sb[:], wo_d.rearrange("(p o n) -> p o n", p=128, o=2))

            # ---- persistent SBUF tensors
            qT = [qkv_pool.tile([128, S], BF16, name=f"qT{p}") for p in range(2)]
            kT = [qkv_pool.tile([128, S], BF16, name=f"kT{p}") for p in range(2)]
            # v65: per (t-chunk, head) a 128-col stationary.
            #   even head (A): [v(64) | ones | 0*63]  -> ctx @ psum 0:64, den @ 64
            #   odd head (B):  [ones | 0*63 | v(64)]  -> den @ psum 0, ctx @ 64:128
            v65 = qkv_pool.tile([128, N_TC, 4, 128], BF16)
            nc.vector.memset(v65[:], 0.0)
            nc.vector.memset(v65[:, :, 0::2, 64:65], 1.0)
            nc.vector.memset(v65[:, :, 1::2, 0:1], 1.0)

            cn = {}  # (st, p) -> cn tile

            # ---------------- emission closures ----------------
            def k_chain(p, kt):
                ps = psa.tile([128, 2 * ST], F32, tag="a", name="ps_k")
                for ec in range(N_EC):
                    nc.tensor.matmul(
                        ps[:, :ST],
                        wk_sb[:, ec, 128 * p : 128 * (p + 1)],
                        xt_sb[:, ec, ST * kt : ST * (kt + 1)],
                        start=(ec == 0),
                        stop=(ec == N_EC - 1),
                    )
                nc.vector.tensor_copy(kT[p][:, ST * kt : ST * (kt + 1)], ps[:, :ST])

            def q_chain(p, st):
                ps = psa.tile([128, 2 * ST], F32, tag="a", name="ps_q")
                for ec in range(N_EC):
                    nc.tensor.matmul(
                        ps[:, :ST],
                        wq_sb[:, ec, 128 * p : 128 * (p + 1)],
                        xt_sb[:, ec, ST * st : ST * (st + 1)],
                        start=(ec == 0),
                        stop=(ec == N_EC - 1),
                    )
                nc.vector.tensor_scalar(
                    qT[p][:, ST * st : ST * (st + 1)],
                    ps[:, :ST],
                    bq_sb[:, p : p + 1],
                    None,
                    ADD,
                )

            def v_chain(tt):
                ps = psa.tile([128, 2 * ST], F32, tag="a", name="ps_v")
                for ec in range(N_EC):
                    nc.tensor.matmul(
                        ps[:, :EL],
                        xt_sb[:, ec, 128 * tt : 128 * (tt + 1)],
                        wv_sb[:, ec, :],
                        start=(ec == 0),
                        stop=(ec == N_EC - 1),
                    )
                vsrc = ps[:, :EL].rearrange("p (h d) -> p h d", h=4)
                # even heads -> cols 0:64, odd heads -> cols 64:128
                nc.vector.tensor_copy(v65[:, tt, 0::2, 0:64], vsrc[:, 0::2, :])
                nc.vector.tensor_copy(v65[:, tt, 1::2, 64:128], vsrc[:, 1::2, :])

            ctx_ps = {}  # (st, p) -> (ctxA tile, ctxB tile)
            ex_tiles = {}  # (st, p, tc) -> ex tile (deleted after use)

            def scores(st, p, tcc):
                sc = psa.tile([128, 2 * ST], F32, tag="a", name="sc")
                nc.tensor.matmul(
                    sc[:, :ST],
                    kT[p][0:64, 128 * tcc : 128 * (tcc + 1)],
                    qT[p][0:64, ST * st : ST * (st + 1)],
                    start=True,
                    stop=True,
                )
                nc.tensor.matmul(
                    sc[:, ST:],
                    kT[p][64:128, 128 * tcc : 128 * (tcc + 1)],
                    qT[p][64:128, ST * st : ST * (st + 1)],
                    start=True,
                    stop=True,
                )
                return sc

            def exp_emit(st, p, tcc, sc):
                ex = ex_pool.tile([128, 2 * ST], BF16, name="ex")
                nc.scalar.activation(ex[:], sc[:], EXP, scale=0.125)
                ex_tiles[(st, p, tcc)] = ex

            def ctx_pair(st, p, tcc):
                if (st, p) not in ctx_ps:
                    a = psc.tile([128, ST], F32, tag="c", name="ctxA")
                    b = psc.tile([128, ST], F32, tag="c", name="ctxB")
                    ctx_ps[(st, p)] = (a, b)
                a, b = ctx_ps[(st, p)]
                ex = ex_tiles.pop((st, p, tcc))
                first, last = tcc == 0, tcc == N_TC - 1
                nc.tensor.matmul(
                    a[:], v65[:, tcc, 2 * p, :], ex[:, :ST], start=first, stop=last
                )
                nc.tensor.matmul(
                    b[:], v65[:, tcc, 2 * p + 1, :], ex[:, ST:], start=first, stop=last
                )

            def den_copies(st, p):
                a, b = ctx_ps[(st, p)]
                dAh = dh_pool.tile([1, ST], FP16, tag="dh", name="dAh")
                nc.vector.tensor_copy(dAh[:], a[64:65, :])
                dBh = dh_pool.tile([1, ST], FP16, tag="dh", name="dBh")
                nc.vector.tensor_copy(dBh[:], b[0:1, :])
                ctx_ps[(st, p)] = (a, b, dAh, dBh)

            def norm(st, p):
                a, b, dAh, dBh = ctx_ps.pop((st, p))
                rbp = psa.tile([128, 2 * ST], F32, tag="a", name="rbp")
                nc.tensor.matmul(
                    rbp[0:64, :ST], ones1[:], dAh[:], start=True, stop=True
                )
                nc.tensor.matmul(
                    rbp[64:128, :ST], ones1[:], dBh[:], start=True, stop=True
                )
                rb32 = rb32_pool.tile([128, ST], F32, name="rb32")
                nc.vector.tensor_copy(rb32[:], rbp[:, :ST])
                rbh = rbh_pool.tile([128, ST], FP16, name="rbh")
                with nc.allow_low_precision(reason="fp16 recip of softmax denom"):
                    nc.vector.reciprocal(rbh[:], rb32[:])
                c = cn_pool.tile([128, ST], BF16, name="cn")
                nc.vector.tensor_tensor(c[0:64, :], a[0:64, :], rbh[0:64, :], MULT)
                nc.vector.tensor_tensor(
                    c[64:128, :], b[64:128, :], rbh[64:128, :], MULT
                )
                nc.vector.tensor_scalar(c[:], c[:], bv_sb[:, p : p + 1], None, ADD)
                cn[(st, p)] = c

            def outproj(st, j):
                ss, nt = j // 2, j % 2
                ps = psa.tile([128, 2 * ST], F32, tag="a", name="ps_o")
                nc.tensor.matmul(
                    ps[:, :ST],
                    cn[(st, 0)][:, 128 * ss : 128 * (ss + 1)],
                    wo_sb[:, 0, ST * nt : ST * (nt + 1)],
                    start=True,
                    stop=False,
                )
                nc.tensor.matmul(
                    ps[:, :ST],
                    cn[(st, 1)][:, 128 * ss : 128 * (ss + 1)],
                    wo_sb[:, 1, ST * nt : ST * (nt + 1)],
                    start=False,
                    stop=True,
                )
                ob = out_pool.tile([128, ST], BF16, name="ob")
                nc.vector.tensor_copy(ob[:], ps[:, :ST])
                srow = ST * st + 128 * ss
                nc.sync.dma_start(
                    out_d[srow : srow + 128, ST * nt : ST * (nt + 1)], ob[:]
                )

            # ---------------- schedule ----------------
            # Injections per loop (st, p), keyed by iteration index.
            def make_fillers():
                F = {(st, p): {i: [] for i in range(N_TC)} for st in range(N_ST)
                     for p in range(2)}
                # kv/q production spread over st0
                F[(0, 0)][0] += [lambda: k_chain(0, 1), lambda: v_chain(1)]
                for i, tt in [(1, 2), (2, 3), (3, 4)]:
                    F[(0, 0)][i] += [lambda t=tt: v_chain(t)]
                F[(0, 0)][4] += [lambda: k_chain(0, 2), lambda: v_chain(5)]
                for i, tt in [(5, 6), (6, 7), (7, 8)]:
                    F[(0, 0)][i] += [lambda t=tt: v_chain(t)]
                F[(0, 0)][8] += [lambda: k_chain(0, 3), lambda: v_chain(9)]
                for i, tt in [(9, 10), (10, 11)]:
                    F[(0, 0)][i] += [lambda t=tt: v_chain(t)]
                F[(0, 0)][11] += [lambda: k_chain(1, 0), lambda: v_chain(12)]
                F[(0, 0)][12] += [lambda: q_chain(1, 0), lambda: v_chain(13)]
                F[(0, 0)][13] += [lambda: k_chain(1, 1), lambda: v_chain(14)]
                F[(0, 0)][14] += [lambda: v_chain(15)]
                F[(0, 1)][0] += [lambda: k_chain(1, 2)]
                F[(0, 1)][2] += [lambda: k_chain(1, 3)]
                # q for next s-tile: paired so the scores-psum ring keeps its
                # 2-iteration lookahead (a lone interposed alloc would reduce
                # it to 1 and stall the Scalar engine by ~0.5us per iteration)
                for st in range(N_ST - 1):
                    F[(st, 1)][10] += [
                        lambda s=st: q_chain(0, s + 1),
                        lambda s=st: q_chain(1, s + 1),
                    ]
                # output projection of s-tile st-1: 2 per iteration, same
                # pairing rationale
                for st in range(1, N_ST):
                    for j in range(8):
                        F[(st, 0)][12 + j // 2] += [
                            lambda s=st - 1, jj=j: outproj(s, jj)
                        ]
                return F

            fillers = make_fillers()
            loops = [(st, p) for st in range(N_ST) for p in range(2)]

            # preamble
            k_chain(0, 0)
            q_chain(0, 0)
            v_chain(0)

            warm_row = const_pool.tile([1, ST], FP16, name="warm_row")
            nc.vector.memset(warm_row[:], 1.0)

            def warm_dummies():
                # Keep the PE's HAM activity window busy through the tail's
                # reciprocal (~4us of otherwise-idle PE would re-throttle the
                # clock gate to 1.2GHz right before the final projection).
                ps = psa.tile([128, 2 * ST], F32, tag="a", name="ps_warm")
                for r in range(8):
                    half = slice(0, 64) if r % 2 == 0 else slice(64, 128)
                    nc.tensor.matmul(
                        ps[half, :ST], ones1[:], warm_row[:], start=True, stop=True
                    )

            carry = []  # closures to inject at the start of the next loop
            for li, (st, p) in enumerate(loops):
                lag = 2 if li == len(loops) - 1 else LAG
                my_fill = fillers[(st, p)]
                for i in range(N_TC):
                    sc = scores(st, p, i)
                    # carried work from the previous loop: ctx tail + den + norm
                    if i < len(carry):
                        carry[i]()
                    for f in my_fill[i]:
                        f()
                    exp_emit(st, p, i, sc)
                    if i >= lag:
                        ctx_pair(st, p, i - lag)
                # build next carry: finish this loop's ctx, den, then norm
                nxt = []
                for tcc in range(N_TC - lag, N_TC):
                    nxt.append(lambda t=tcc, s=st, q=p: ctx_pair(s, q, t))
                nxt.append(lambda s=st, q=p: den_copies(s, q))
                nxt.append(lambda s=st, q=p: norm(s, q))
                carry = nxt

            # tail: flush the last carry (ctx tail, den, norm of (3,1)) with
            # warm-keeper dummies over the reciprocal window, then the final
            # output projection
            for f in carry:
                f()
            warm_dummies()
            for j in range(8):
                outproj(N_ST - 1, j)
    _split_multi_waits(nc)
    return nc


_NC = None


def _get_nc():
    global _NC
    if _NC is None:
        _NC = build_bass()
    return _NC


def make_in_maps(hidden_states, Wq, bq, Wk, bk, Wv, bv, Wo):
    """Host-side sharding/layout prep. Returns list of 8 per-core input dicts."""
    hs = np.asarray(hidden_states, dtype=np.float32)
    Wq = np.asarray(Wq, dtype=np.float32)
    Wk = np.asarray(Wk, dtype=np.float32)
    Wv = np.asarray(Wv, dtype=np.float32)
    Wo = np.asarray(Wo, dtype=np.float32)
    bq = np.asarray(bq, dtype=np.float32)
    bv = np.asarray(bv, dtype=np.float32)

    # xt host layout: [p, quarter, o, s] flattened -> every xt DMA is one
    # contiguous 8KB-per-partition transfer
    xt = [
        np.ascontiguousarray(
            hs[b].T.reshape(N_EC, 128, 4, ST).transpose(1, 2, 0, 3)
        ).astype(BF16_NP).reshape(-1)
        for b in range(B)
    ]
    in_maps = []
    for c in range(N_CORES):
        b, g = divmod(c, N_CORES // B)
        h0 = HEADS_PER_CORE * g
        hsl = slice(h0, h0 + HEADS_PER_CORE)
        # [H_loc, E, DH] -> [E, H_loc*DH] head-major columns
        def wlay(W):  # [E, EL] -> [p, o, d] flattened (contiguous DMA)
            return np.ascontiguousarray(
                W.reshape(N_EC, 128, EL).transpose(1, 0, 2)
            ).astype(BF16_NP).reshape(-1)

        wq_c = wlay(Wq[hsl].transpose(1, 0, 2).reshape(E, EL))
        wk_c = wlay(Wk[hsl].transpose(1, 0, 2).reshape(E, EL))
        wv_c = wlay(Wv[hsl].transpose(1, 0, 2).reshape(E, EL))
        wo_c = np.ascontiguousarray(
            Wo[EL * g : EL * (g + 1), :].reshape(2, 128, E).transpose(1, 0, 2)
        ).astype(BF16_NP).reshape(-1)
        bq_c = np.ascontiguousarray(bq[hsl].reshape(EL).reshape(2, 128).T)
        bv_c = np.ascontiguousarray(bv[hsl].reshape(EL).reshape(2, 128).T)
        in_maps.append(
            {
                "xt": xt[b],
                "wq": wq_c,
                "wk": wk_c,
                "wv": wv_c,
                "wo": wo_c,
                "bq2": bq_c,
                "bv2": bv_c,
            }
        )
    return in_maps


def kernel(hidden_states, mask, Wq, bq, Wk, bk, Wv, bv, Wo, bo, **run_kwargs):
    """Full-input entry point. mask is all-ones per the problem spec (ignored)."""
    nc = _get_nc()
    in_maps = make_in_maps(hidden_states, Wq, bq, Wk, bk, Wv, bv, Wo)
    res = run_bass_kernel_spmd(nc, in_maps, core_ids=list(range(N_CORES)), **run_kwargs)
    bo = np.asarray(bo, dtype=np.float32)
    out = np.zeros((B, S, E), dtype=np.float32)
    for c in range(N_CORES):
        out[c // (N_CORES // B)] += res.results[c]["out"].astype(np.float32)
    out += bo
    kernel.last_results = res
    return out
